# revision 1
# baseline (speedup 1.0000x reference)
"""Trainium2 Bass kernel for causal sliding-window self-attention.

Shapes (hardcoded): B=2, T=2048, NH=12, HD=128, HIDDEN=1536, window=1024.

Sharding: 8 cores; core c handles batch b=c//4 and heads [3*(c%4), 3*(c%4)+3).
Each core computes q/k/v projections for its 3 heads (contraction over the
full hidden dim), RoPE + RMS-norm, block-sparse attention (query block i
attends key blocks [i-8, i]), and a partial output projection. The host sums
the 4 partial projections per batch element. No collectives.

Layout strategy per head:
  - project q,k,v into [T, d] (token-major) so RoPE/RMS-norm reductions are
    free-dim reductions; PE-transpose q,k to [d, T] (fp16, 2-wait-slot safe)
  - scores computed transposed: s_t[c, r] = k_tile.T @ q_pair  (so p @ v needs
    no transpose of p);  q pre-scaled by rms_q, k pre-scaled by rms_k/sqrt(HD)
  - softmax denominator: a ones-column appended to v accumulates sum(p) in the
    same PSUM tile as p@v
  - exp computed as exp(s - 3) (softmax shift-invariant) to keep fp16 p finite
"""

import sys
import os

sys.path.insert(0, "/opt/trn_rl_repo")

import numpy as np
from contextlib import ExitStack

import concourse.bass as bass
import concourse.bacc as bacc
import concourse.tile as tile
from concourse import mybir
from concourse.bass_utils import run_bass_kernel_spmd

F32 = mybir.dt.float32
F16 = mybir.dt.float16
AF = mybir.ActivationFunctionType

B, T, NH, HD = 2, 2048, 12, 128
HIDDEN = NH * HD
EPS = 1.1920928955078125e-07
NB = T // 128        # 16 token blocks
KT = HIDDEN // 128   # 12 contraction tiles
WB = 8               # window in blocks (1024/128)
NHC = 3              # heads per core
EXP_SHIFT = -3.0     # exp(s + EXP_SHIFT); cancels in softmax, keeps fp16 finite

_cached_nc = None


def _window(i):
    return list(range(max(0, i - WB), i + 1))


def _build(stages="ABC", cfg=None):
    cfg = dict(cfg) if cfg else {}
    cfg.setdefault("pipe", 0)
    nc = bacc.Bacc("TRN2", target_bir_lowering=False, debug=False, num_devices=8)

    x16 = nc.dram_tensor("x16", [HIDDEN, T], F16, kind="ExternalInput")
    wq16 = nc.dram_tensor("wq16", [HIDDEN, NHC * HD], F16, kind="ExternalInput")
    wk16 = nc.dram_tensor("wk16", [HIDDEN, NHC * HD], F16, kind="ExternalInput")
    wv16 = nc.dram_tensor("wv16", [HIDDEN, NHC * HD], F16, kind="ExternalInput")
    wp16 = nc.dram_tensor("wp16", [NHC * HD, HIDDEN], F16, kind="ExternalInput")
    cos3 = nc.dram_tensor("cos3", [T, NHC * 64], F32, kind="ExternalInput")
    sin3 = nc.dram_tensor("sin3", [T, NHC * 64], F32, kind="ExternalInput")
    mdiag = nc.dram_tensor("mdiag", [128, 128], F16, kind="ExternalInput")
    medge = nc.dram_tensor("medge", [128, 128], F16, kind="ExternalInput")
    ident = nc.dram_tensor("ident", [128, 128], F16, kind="ExternalInput")
    outp = nc.dram_tensor("outp", [T, HIDDEN], F32, kind="ExternalOutput")

    with tile.TileContext(nc) as tc:
        with ExitStack() as ctx:
            const = ctx.enter_context(tc.tile_pool(name="const", bufs=1))
            persist = ctx.enter_context(tc.tile_pool(name="persist", bufs=1))

            # --- constants / weights -------------------------------------
            wqt = const.tile([128, KT, NHC * HD], F16)
            wkt = const.tile([128, KT, NHC * HD], F16)
            wvt = const.tile([128, KT, NHC * HD], F16)
            wpt = const.tile([128, NHC, HIDDEN], F16)
            nc.sync.dma_start(wqt[:], wq16.ap().rearrange("(k p) n -> p k n", p=128))
            nc.sync.dma_start(wkt[:], wk16.ap().rearrange("(k p) n -> p k n", p=128))
            nc.sync.dma_start(wvt[:], wv16.ap().rearrange("(k p) n -> p k n", p=128))
            nc.sync.dma_start(wpt[:], wp16.ap().rearrange("(k p) n -> p k n", p=128))
            cost = const.tile([128, NB, NHC, 64], F32)
            sint = const.tile([128, NB, NHC, 64], F32)
            nc.sync.dma_start(
                cost[:], cos3.ap().rearrange("(m p) (h c) -> p m h c", p=128, h=NHC)
            )
            nc.sync.dma_start(
                sint[:], sin3.ap().rearrange("(m p) (h c) -> p m h c", p=128, h=NHC)
            )
            mdg = const.tile([128, 128], F16)
            medg = const.tile([128, 128], F16)
            idt = const.tile([128, 128], F16)
            nc.sync.dma_start(mdg[:], mdiag.ap())
            nc.sync.dma_start(medg[:], medge.ap())
            nc.sync.dma_start(idt[:], ident.ap())
            biast = const.tile([128, 4], F32)
            nc.vector.memset(biast[:, 0:1], EPS)
            nc.vector.memset(biast[:, 1:2], HD * EPS)
            nc.vector.memset(biast[:, 2:3], EXP_SHIFT)
            b_eps = biast[:, 0:1]
            b_epsk = biast[:, 1:2]
            b_shift = biast[:, 2:3]

            # --- persistent per-head tensors -----------------------------
            qt = persist.tile([128, NHC, T], F16)   # q^T  [d, t] per head
            kt = persist.tile([128, NHC, T], F16)   # k^T  [d, t] per head
            yt = persist.tile([128, NHC, T], F16)   # y^T  [d, t] per head
            vext = persist.tile([128, NHC, NB, 132], F16)  # v tiles + ones col
            nc.gpsimd.memset(vext[:], 1.0)  # col 128 stays 1.0; 0:128 overwritten

            x16r = x16.ap().rearrange("(k p) t -> p k t", p=128)

            # pools for the fused loop
            xp = ctx.enter_context(tc.tile_pool(name="xp", bufs=cfg.get("xp", 4)))
            rp = ctx.enter_context(tc.tile_pool(name="rp", bufs=cfg.get("rp", 3)))
            pp = ctx.enter_context(tc.tile_pool(name="pp", bufs=cfg.get("pp", 12)))
            yp = ctx.enter_context(tc.tile_pool(name="yp", bufs=cfg.get("yp", 3)))
            op_sb = ctx.enter_context(tc.tile_pool(name="opsb", bufs=cfg.get("osb", 3)))
            psA = ctx.enter_context(
                tc.tile_pool(name="psA", bufs=cfg.get("psA", 1), space="PSUM")
            )
            tpps = ctx.enter_context(
                tc.tile_pool(name="tpps", bufs=cfg.get("tpps", 1), space="PSUM")
            )
            spsum = ctx.enter_context(
                tc.tile_pool(name="spsum", bufs=cfg.get("sps", 2), space="PSUM")
            )
            opsum = ctx.enter_context(
                tc.tile_pool(name="opsum", bufs=cfg.get("ops", 1), space="PSUM")
            )
            cps = ctx.enter_context(
                tc.tile_pool(name="cps", bufs=cfg.get("cps", 1), space="PSUM")
            )

            def stage_a(m):
                xm = xp.tile([128, KT, 128], F16, tag="xm")
                nc.sync.dma_start(xm[:], x16r[:, :, m * 128 : (m + 1) * 128])
                psq = psA.tile([128, NHC, HD], F32, tag="psq")
                psk = psA.tile([128, NHC, HD], F32, tag="psk")
                psv = psA.tile([128, NHC, HD], F32, tag="psv")
                if cfg.get("seq", 0):
                    for ps_, wt_ in ((psq, wqt), (psk, wkt), (psv, wvt)):
                        for kk in range(KT):
                            nc.tensor.matmul(
                                ps_[:], xm[:, kk, :], wt_[:, kk, :],
                                start=(kk == 0), stop=(kk == KT - 1),
                            )
                else:
                    for kk in range(KT):
                        nc.tensor.matmul(
                            psq[:], xm[:, kk, :], wqt[:, kk, :],
                            start=(kk == 0), stop=(kk == KT - 1),
                        )
                        nc.tensor.matmul(
                            psk[:], xm[:, kk, :], wkt[:, kk, :],
                            start=(kk == 0), stop=(kk == KT - 1),
                        )
                        nc.tensor.matmul(
                            psv[:], xm[:, kk, :], wvt[:, kk, :],
                            start=(kk == 0), stop=(kk == KT - 1),
                        )
                cosm = cost[:, m]  # [128, 3, 64]
                sinm = sint[:, m]
                for src, b_rms, scale_rms, dst in (
                    (psq, b_eps, 1.0 / HD, qt),
                    (psk, b_epsk, 1.0, kt),
                ):
                    t1 = rp.tile([128, NHC, 64], F32, tag="t1")
                    t2 = rp.tile([128, NHC, 64], F32, tag="t2")
                    t3 = rp.tile([128, NHC, 64], F32, tag="t3")
                    t4 = rp.tile([128, NHC, 64], F32, tag="t4")
                    u = rp.tile([128, NHC, HD], F32, tag="u")
                    nc.vector.tensor_mul(t1[:], src[:, :, 0:64], cosm)
                    nc.vector.tensor_mul(t2[:], src[:, :, 64:128], sinm)
                    nc.vector.tensor_add(u[:, :, 0:64], t1[:], t2[:])
                    nc.vector.tensor_mul(t3[:], src[:, :, 64:128], cosm)
                    nc.vector.tensor_mul(t4[:], src[:, :, 0:64], sinm)
                    nc.vector.tensor_sub(u[:, :, 64:128], t3[:], t4[:])
                    # rms sums of squares from rotated u (rope preserves norm)
                    sqs = rp.tile([128, 4], F32, tag="sqs")
                    scq = rp.tile([128, NHC, HD], F32, tag="scq")
                    nc.vector.tensor_mul(scq[:], u[:], u[:])
                    nc.vector.tensor_reduce(
                        sqs[:, 0:3], scq[:],
                        mybir.AxisListType.X, mybir.AluOpType.add,
                    )
                    root = rp.tile([128, 4], F32, tag="root")
                    nc.scalar.activation(
                        root[:, 0:3], sqs[:, 0:3], AF.Sqrt,
                        bias=b_rms, scale=scale_rms,
                    )
                    rs = rp.tile([128, 4], F32, tag="rs")
                    nc.vector.reciprocal(rs[:, 0:3], root[:, 0:3])
                    rq = rp.tile([128, NHC, HD], F16, tag="rq")
                    for hh in range(NHC):
                        nc.vector.tensor_scalar_mul(
                            rq[:, hh, :], u[:, hh, :], rs[:, hh : hh + 1]
                        )
                    for hh in range(NHC):
                        if cfg.get("dmat", 0):
                            nc.sync.dma_start(
                                dst[:, hh, m * 128 : (m + 1) * 128],
                                rq[:, hh, :], transpose=True,
                            )
                        else:
                            tp = tpps.tile([128, 128], F16, tag="tp")
                            nc.tensor.transpose(tp[:], rq[:, hh, :], idt[:])
                            nc.vector.tensor_copy(
                                dst[:, hh, m * 128 : (m + 1) * 128], tp[:]
                            )
                for hh in range(NHC):
                    nc.vector.tensor_copy(vext[:, hh, m, 0:128], psv[:, hh, :])

            def attention_pair(pr):
                i0, i1 = 2 * pr, 2 * pr + 1
                js = list(range(max(0, i0 - WB), i1 + 1))
                for hh in range(NHC):
                    ptloc = {}
                    for g0 in range(0, len(js), 2):
                        grp = js[g0 : g0 + 2]
                        w = len(grp) * 256
                        sps = spsum.tile([128, 512], F32, tag="sps")
                        for gi, j in enumerate(grp):
                            nc.tensor.matmul(
                                sps[:, gi * 256 : (gi + 1) * 256],
                                kt[:, hh, j * 128 : (j + 1) * 128],
                                qt[:, hh, i0 * 128 : (i0 + 2) * 128],
                                start=True, stop=True,
                            )
                        ptile = pp.tile([128, 512], F16, tag="pt")
                        nc.scalar.activation(
                            ptile[:, 0:w], sps[:, 0:w], AF.Exp,
                            bias=b_shift, scale=1.0,
                        )
                        for gi, j in enumerate(grp):
                            ptloc[j] = (ptile, gi * 256)

                    def mask_mult(j, half, mask):
                        t, off = ptloc[j]
                        o = off + half * 128
                        nc.gpsimd.tensor_mul(
                            t[:, o : o + 128], t[:, o : o + 128], mask[:]
                        )

                    mask_mult(i0, 0, mdg)
                    mask_mult(i1, 1, mdg)
                    if i0 >= WB:
                        mask_mult(i0 - WB, 0, medg)
                    if i1 >= WB:
                        mask_mult(i1 - WB, 1, medg)

                    for half, i in enumerate((i0, i1)):
                        jsi = _window(i)
                        ops = opsum.tile([128, 132], F32, tag="ops")
                        for idx, j in enumerate(jsi):
                            t, off = ptloc[j]
                            o = off + half * 128
                            nc.tensor.matmul(
                                ops[:, 0:129],
                                t[:, o : o + 128],
                                vext[:, hh, j, 0:129],
                                start=(idx == 0), stop=(idx == len(jsi) - 1),
                            )
                        rden = yp.tile([128, 1], F32, tag="rden")
                        nc.vector.reciprocal(rden[:], ops[:, 128:129])
                        ysb = yp.tile([128, 128], F16, tag="ysb")
                        nc.vector.tensor_scalar_mul(ysb[:], ops[:, 0:128], rden[:])
                        if cfg.get("dmat", 0):
                            nc.sync.dma_start(
                                yt[:, hh, i * 128 : (i + 1) * 128],
                                ysb[:], transpose=True,
                            )
                        else:
                            ytp = tpps.tile([128, 128], F16, tag="tp")
                            nc.tensor.transpose(ytp[:], ysb[:], idt[:])
                            nc.vector.tensor_copy(
                                yt[:, hh, i * 128 : (i + 1) * 128], ytp[:]
                            )

            def out_proj(m):
                osb = op_sb.tile([128, HIDDEN], F32, tag="osb")
                for n in range(3):
                    po = cps.tile([128, 512], F32, tag="po")
                    for hh in range(NHC):
                        nc.tensor.matmul(
                            po[:],
                            yt[:, hh, m * 128 : (m + 1) * 128],
                            wpt[:, hh, n * 512 : (n + 1) * 512],
                            start=(hh == 0), stop=(hh == NHC - 1),
                        )
                    if n % 2 == 0:
                        nc.vector.tensor_copy(osb[:, n * 512 : (n + 1) * 512], po[:])
                    else:
                        nc.scalar.copy(osb[:, n * 512 : (n + 1) * 512], po[:])
                nc.sync.dma_start(outp.ap()[m * 128 : (m + 1) * 128, :], osb[:])

            # fused pair-major schedule (optionally software-pipelined by one
            # pair: attention/out-proj for pair pr-1 run between A-pairs)
            nreps = cfg.get("reps", 0)
            repctx = tc.For_i(0, nreps, 1) if nreps else None
            if repctx is not None:
                repctx.__enter__()
            if cfg.get("pipe", 0):
                stage_a(0)
                stage_a(1)
                for pr in range(1, NB // 2):
                    stage_a(2 * pr)
                    stage_a(2 * pr + 1)
                    attention_pair(pr - 1)
                    out_proj(2 * pr - 2)
                    out_proj(2 * pr - 1)
                attention_pair(NB // 2 - 1)
                out_proj(NB - 2)
                out_proj(NB - 1)
            else:
                for pr in range(NB // 2):
                    stage_a(2 * pr)
                    stage_a(2 * pr + 1)
                    attention_pair(pr)
                    out_proj(2 * pr)
                    out_proj(2 * pr + 1)
            if repctx is not None:
                repctx.__exit__(None, None, None)

    nc.compile()
    return nc


def _get_nc():
    global _cached_nc
    if _cached_nc is None:
        _cached_nc = _build()
    return _cached_nc


def _rope_tables():
    d_half = HD // 2
    inv = 1.0 / (10000.0 ** (np.arange(d_half, dtype=np.float64) / d_half))
    t = np.arange(T, dtype=np.float64)
    f = t[:, None] * inv[None, :]
    return np.cos(f), np.sin(f)


def kernel(x, cos, sin, Wq, Wk, Wv, Wp, window, _trace=False, _result_holder=None):
    x = np.asarray(x, dtype=np.float32)
    cos = np.asarray(cos, dtype=np.float32)
    sin = np.asarray(sin, dtype=np.float32)
    Wq = np.asarray(Wq, dtype=np.float32)
    Wk = np.asarray(Wk, dtype=np.float32)
    Wv = np.asarray(Wv, dtype=np.float32)
    Wp = np.asarray(Wp, dtype=np.float32)
    assert int(window) == 1024, f"kernel hardcodes window=1024, got {window}"

    cosn = np.ascontiguousarray(cos[0, :, 0, :])  # [T, 64]
    sinn = np.ascontiguousarray(sin[0, :, 0, :])
    cos3 = np.tile(cosn, (1, NHC)).astype(np.float32)  # [T, 192]
    sin3 = np.tile(sinn, (1, NHC)).astype(np.float32)

    c = np.arange(128)[:, None]
    r = np.arange(128)[None, :]
    mdiag = (c <= r).astype(np.float16)
    medge = (r <= c).astype(np.float16)
    ident = np.eye(128, dtype=np.float16)

    in_maps = []
    for core in range(8):
        b = core // 4
        h0 = NHC * (core % 4)
        S = slice(h0 * HD, (h0 + NHC) * HD)
        in_maps.append(
            {
                "x16": np.ascontiguousarray(x[b].T).astype(np.float16),
                "wq16": np.ascontiguousarray(Wq[S, :].T).astype(np.float16),
                "wk16": np.ascontiguousarray(Wk[S, :].T).astype(np.float16),
                "wv16": np.ascontiguousarray(Wv[S, :].T).astype(np.float16),
                "wp16": np.ascontiguousarray(Wp[:, S].T).astype(np.float16),
                "cos3": cos3,
                "sin3": sin3,
                "mdiag": mdiag,
                "medge": medge,
                "ident": ident,
            }
        )

    nc = _get_nc()
    res = run_bass_kernel_spmd(nc, in_maps, list(range(8)), trace=_trace)
    if _result_holder is not None:
        _result_holder.append(res)

    out = np.zeros((B, T, HIDDEN), dtype=np.float32)
    for core in range(8):
        out[core // 4] += res.results[core]["outp"]
    return out

